# revision 4
# baseline (speedup 1.0000x reference)
"""MoE MLP (top-2 routing, 8 experts) on 8 Trainium2 NeuronCores.

Strategy (expert-parallel, per the sharding hint): each core owns one
expert's weights. The router (a [8,1024] matmul + softmax + top-2 —
0.05% of total FLOPs) runs on the host, which doubles as the dispatch
step: tokens are gathered per selected expert and shipped to that
expert's core, replacing the all-to-all. Each core runs a fused
gelu-MLP Bass kernel over its routed tokens:

    yT = w ⊙ (W_out^T @ gelu(W_in^T @ xT + b_in) + b_out)

in a transposed layout (tokens along the free axis) so both matmuls
keep the *weights* stationary on the PE array and no on-chip
transposes are needed anywhere. W_out stays resident in SBUF; W_in
streams once per token chunk. The host scatter-adds the per-expert
results back into the full [B,S,D] output.

Matmuls run in fp16 (same PE throughput as bf16 — 4x fp32 — but 8x
finer mantissa; measured end-to-end error vs the fp32 reference is
~4e-4 scale-relative). Set MOE_PREC=fp32 to force full fp32 matmuls.
"""

import contextlib
import ctypes
import os
import sys
import types
from contextlib import ExitStack

import numpy as np

import concourse.bass as bass
import concourse.mybir as mybir
import concourse.tile as tile
from concourse import bacc
from concourse.bass_utils import run_bass_kernel_spmd


def _install_ntff_hook():
    """Provide antenv.axon_hooks (absent in this image) so BASS_TRACE=1
    can capture NTFF profiles through the axon PJRT .so. No-op if the
    module already exists or the .so/symbols are unavailable."""
    try:
        from antenv.axon_hooks import get_axon_ntff_profile_hook  # noqa: F401
        return
    except ImportError:
        pass
    so_path = "/opt/axon/libaxon_pjrt.so"
    if not os.path.exists(so_path):
        return
    try:
        lib = ctypes.CDLL(so_path)
    except OSError:
        return
    if not hasattr(lib, "axon_start_nrt_profile"):
        return
    lib.axon_start_nrt_profile.argtypes = [
        ctypes.POINTER(ctypes.c_int64), ctypes.c_size_t]
    lib.axon_start_nrt_profile.restype = ctypes.c_int64
    lib.axon_stop_nrt_profile.argtypes = [ctypes.c_char_p]
    lib.axon_stop_nrt_profile.restype = ctypes.c_int64

    @contextlib.contextmanager
    def _hook(output_dir, device_ids):
        import jax
        jax.devices()  # force PJRT init so the .so's client exists
        if device_ids:
            ids = (ctypes.c_int64 * len(device_ids))(*device_ids)
            rc = lib.axon_start_nrt_profile(ids, len(device_ids))
        else:
            rc = lib.axon_start_nrt_profile(None, 0)
        if rc != 0:
            raise RuntimeError(f"axon_start_nrt_profile rc={rc}")
        try:
            yield
        finally:
            n = lib.axon_stop_nrt_profile(str(output_dir).encode())
            print(f"ntff profile: {n} file(s) -> {output_dir}", file=sys.stderr)

    import antenv
    mod = types.ModuleType("antenv.axon_hooks")
    mod.get_axon_ntff_profile_hook = lambda: _hook
    mod.set_axon_ntff_profile_hook = lambda h: None
    sys.modules["antenv.axon_hooks"] = mod
    antenv.axon_hooks = mod

B, S, D, F, E = 4, 2048, 1024, 4096, 8
T = B * S
TOP_K = 2
NCORES = 8
P = 128
ND, NF = D // P, F // P  # 8, 32

# test.py pokes these for profiling info
LAST_RESULT = None

_cache = {}


def _chunk_list(C):
    """Token chunks (PSUM free-dim <= 512, multiples of 128)."""
    chunks = [512] * (C // 512)
    if C % 512:
        chunks.append(C % 512)
    return chunks


def _build_bass(C, prec):
    dt = mybir.dt
    fp16_path = prec != "fp32"
    io_dt = dt.float16 if fp16_path else dt.float32
    nc = bacc.Bacc("TRN2", target_bir_lowering=False, debug=False)

    xT = nc.dram_tensor("xT", [D, C], io_dt, kind="ExternalInput")
    win = nc.dram_tensor("win", [D, F], io_dt, kind="ExternalInput")
    wout = nc.dram_tensor("wout", [F, D], io_dt, kind="ExternalInput")
    bin_ = nc.dram_tensor("bin", [F], dt.float32, kind="ExternalInput")
    bout = nc.dram_tensor("bout", [D], dt.float32, kind="ExternalInput")
    wcomb = nc.dram_tensor("wcomb", [P, C], dt.float32, kind="ExternalInput")
    yT = nc.dram_tensor("yT", [D, C], dt.float32, kind="ExternalOutput")

    xT_r = xT.ap().rearrange("(dn p) c -> p dn c", p=P)
    win_r = win.ap().rearrange("(dn p) f -> p dn f", p=P)
    wout_r = wout.ap().rearrange("(fn p) d -> p fn d", p=P)
    yT_r = yT.ap().rearrange("(dn p) c -> p dn c", p=P)

    chunks = _chunk_list(C)

    with tile.TileContext(nc) as tc, ExitStack() as ctx:
        consts = ctx.enter_context(tc.tile_pool(name="consts", bufs=1))
        xpool = ctx.enter_context(tc.tile_pool(name="x", bufs=2))
        winpool = ctx.enter_context(tc.tile_pool(name="win", bufs=3))
        woutpool = ctx.enter_context(tc.tile_pool(name="wout", bufs=1))
        hpool = ctx.enter_context(tc.tile_pool(name="h", bufs=1))
        ypool = ctx.enter_context(tc.tile_pool(name="y", bufs=4))
        psum_h = ctx.enter_context(tc.tile_pool(name="ph", bufs=4, space="PSUM"))
        psum_y = ctx.enter_context(tc.tile_pool(name="py", bufs=2, space="PSUM"))

        bin_t = consts.tile([P, NF], dt.float32)
        nc.sync.dma_start(bin_t[:], bin_.ap().rearrange("(fo fi) -> fi fo", fi=P))
        bout_t = consts.tile([P, ND], dt.float32)
        nc.sync.dma_start(bout_t[:], bout.ap().rearrange("(do di) -> di do", di=P))
        w_t = consts.tile([P, C], dt.float32)
        nc.sync.dma_start(w_t[:], wcomb.ap())

        if fp16_path:
            # whole W_out resident in SBUF (8.4 MB fp16), loaded once
            wout_tiles = []
            for fo in range(8):
                wt = woutpool.tile([P, 4, D], io_dt, tag=f"wout{fo}")
                nc.sync.dma_start(wt[:], wout_r[:, fo * 4:(fo + 1) * 4, :])
                wout_tiles.append(wt)

        off = 0
        for ck in chunks:
            csl = slice(off, off + ck)
            x_t = xpool.tile([P, ND, ck], io_dt, tag="x")
            nc.sync.dma_start(x_t[:], xT_r[:, :, csl])

            # ---- phase A: h = gelu(W_in^T @ x + b_in), laid out [f, tok]
            h_t = hpool.tile([P, NF, ck], io_dt, tag="h")
            for fo in range(8):  # 512-wide stripes of F
                win_t = winpool.tile([P, ND, 512], io_dt, tag="win")
                nc.sync.dma_start(win_t[:], win_r[:, :, fo * 512:(fo + 1) * 512])
                for j in range(4):
                    fc = fo * 4 + j
                    ph = psum_h.tile([P, ck], dt.float32, tag="ph")
                    for dn in range(ND):
                        nc.tensor.matmul(
                            ph[:],
                            win_t[:, dn, j * P:(j + 1) * P],
                            x_t[:, dn, :],
                            start=(dn == 0),
                            stop=(dn == ND - 1),
                        )
                    nc.scalar.activation(
                        h_t[:, fc, :], ph[:],
                        mybir.ActivationFunctionType.Gelu,
                        bias=bin_t[:, fc:fc + 1],
                    )

            # ---- phase B: y = w * (W_out^T @ h + b_out), laid out [d, tok]
            if fp16_path:
                for dn in range(ND):
                    py = psum_y.tile([P, ck], dt.float32, tag="py")
                    for fc in range(NF):
                        nc.tensor.matmul(
                            py[:],
                            wout_tiles[fc // 4][:, fc % 4, dn * P:(dn + 1) * P],
                            h_t[:, fc, :],
                            start=(fc == 0),
                            stop=(fc == NF - 1),
                        )
                    y_t = ypool.tile([P, ck], dt.float32, tag="y")
                    nc.scalar.activation(
                        y_t[:], py[:],
                        mybir.ActivationFunctionType.Identity,
                        bias=bout_t[:, dn:dn + 1],
                    )
                    nc.vector.tensor_mul(y_t[:], y_t[:], w_t[:, csl])
                    nc.sync.dma_start(yT_r[:, dn, csl], y_t[:])
            else:
                # fp32: W_out too big to keep resident; stream it per chunk
                # in two d-halves (4 PSUM banks live per half).
                for dh in range(2):
                    pys = []
                    for i in range(4):
                        py = psum_y.tile([P, ck], dt.float32, tag=f"py{i}")
                        pys.append(py)
                    for fc in range(NF):
                        wt = woutpool.tile([P, 512], io_dt, tag="wouts")
                        nc.sync.dma_start(
                            wt[:], wout_r[:, fc, dh * 512:(dh + 1) * 512])
                        for i in range(4):
                            nc.tensor.matmul(
                                py := pys[i],
                                wt[:, i * P:(i + 1) * P],
                                h_t[:, fc, :],
                                start=(fc == 0),
                                stop=(fc == NF - 1),
                            )
                    for i in range(4):
                        dn = dh * 4 + i
                        y_t = ypool.tile([P, ck], dt.float32, tag="y")
                        nc.scalar.activation(
                            y_t[:], pys[i][:],
                            mybir.ActivationFunctionType.Identity,
                            bias=bout_t[:, dn:dn + 1],
                        )
                        nc.vector.tensor_mul(y_t[:], y_t[:], w_t[:, csl])
                        nc.sync.dma_start(yT_r[:, dn, csl], y_t[:])
            off += ck

    nc.compile()
    return nc


def _get_nc(C, prec):
    key = (C, prec)
    if key not in _cache:
        _cache[key] = _build_bass(C, prec)
    return _cache[key]


def _route(x, W_router):
    """Host-side router: top-2 selection + renormalized weights (fp64).

    Matches jax.lax.top_k on softmax(logits): softmax is monotone so
    top-2 of logits is identical, with ties broken toward lower index
    (argsort stable on -logits).
    """
    lg = x.astype(np.float64) @ W_router.T.astype(np.float64)
    top2 = np.argsort(-lg, axis=1, kind="stable")[:, :TOP_K]
    l1 = np.take_along_axis(lg, top2[:, 0:1], 1)
    l2 = np.take_along_axis(lg, top2[:, 1:2], 1)
    e2 = np.exp(l2 - l1)
    w1 = (1.0 / (1.0 + e2)).astype(np.float32)
    w2 = (e2 / (1.0 + e2)).astype(np.float32)
    return top2, np.concatenate([w1, w2], axis=1)


def kernel(residual, W_router, W_in, b_in, W_out, b_out):
    global LAST_RESULT
    prec = os.environ.get("MOE_PREC", "fp16")
    np_io = np.float16 if prec != "fp32" else np.float32

    x = np.ascontiguousarray(np.asarray(residual, dtype=np.float32).reshape(T, D))
    W_in = np.asarray(W_in, dtype=np.float32)
    W_out = np.asarray(W_out, dtype=np.float32)
    b_in = np.asarray(b_in, dtype=np.float32)
    b_out = np.asarray(b_out, dtype=np.float32)

    top2, wts = _route(x, np.asarray(W_router, dtype=np.float32))

    idxs, ws = [], []
    for e in range(E):
        sel0 = top2[:, 0] == e
        sel1 = top2[:, 1] == e
        idx = np.concatenate([np.where(sel0)[0], np.where(sel1)[0]])
        w = np.concatenate([wts[sel0, 0], wts[sel1, 1]])
        idxs.append(idx)
        ws.append(w)

    C = max(len(i) for i in idxs)
    C = ((C + P - 1) // P) * P
    nc = _get_nc(C, prec)

    xt = np.ascontiguousarray(x.T)  # [D, T]
    in_maps = []
    for e in range(E):
        cnt = len(idxs[e])
        xT_e = np.zeros((D, C), dtype=np_io)
        xT_e[:, :cnt] = xt[:, idxs[e]]
        wc_e = np.zeros((P, C), dtype=np.float32)
        wc_e[:, :cnt] = ws[e][None, :]
        in_maps.append({
            "xT": xT_e,
            "win": np.ascontiguousarray(W_in[e], dtype=np_io),
            "wout": np.ascontiguousarray(W_out[e], dtype=np_io),
            "bin": b_in[e],
            "bout": b_out[e],
            "wcomb": wc_e,
        })

    if os.environ.get("BASS_TRACE"):
        _install_ntff_hook()
    LAST_RESULT = run_bass_kernel_spmd(nc, in_maps, list(range(NCORES)))

    y = np.zeros((T, D), dtype=np.float32)
    for e in range(E):
        cnt = len(idxs[e])
        y[idxs[e]] += LAST_RESULT.results[e]["yT"][:, :cnt].T
    return y.reshape(B, S, D)


# revision 6
# speedup vs baseline: 1.0222x; 1.0222x over previous
"""MoE MLP (top-2 routing, 8 experts) on 8 Trainium2 NeuronCores.

Strategy (expert-parallel, per the sharding hint): each core owns one
expert's weights. The router (a [8,1024] matmul + softmax + top-2 —
0.05% of total FLOPs) runs on the host, which doubles as the dispatch
step: tokens are gathered per selected expert and shipped to that
expert's core, replacing the all-to-all. Each core runs a fused
gelu-MLP Bass kernel over its routed tokens:

    yT = w ⊙ (W_out^T @ gelu(W_in^T @ xT + b_in) + b_out)

in a transposed layout (tokens along the free axis) so both matmuls
keep the *weights* stationary on the PE array and no on-chip
transposes are needed anywhere. W_out stays resident in SBUF; W_in
streams once per token chunk. The host scatter-adds the per-expert
results back into the full [B,S,D] output.

Matmuls run in fp16 (same PE throughput as bf16 — 4x fp32 — but 8x
finer mantissa; measured end-to-end error vs the fp32 reference is
~4e-4 scale-relative). Set MOE_PREC=fp32 to force full fp32 matmuls.
"""

import contextlib
import ctypes
import os
import sys
import types
from contextlib import ExitStack

import numpy as np

import concourse.bass as bass
import concourse.mybir as mybir
import concourse.tile as tile
from concourse import bacc
from concourse.bass_utils import run_bass_kernel_spmd


def _install_ntff_hook():
    """Provide antenv.axon_hooks (absent in this image) so BASS_TRACE=1
    can capture NTFF profiles through the axon PJRT .so. No-op if the
    module already exists or the .so/symbols are unavailable."""
    try:
        from antenv.axon_hooks import get_axon_ntff_profile_hook  # noqa: F401
        return
    except ImportError:
        pass
    so_path = "/opt/axon/libaxon_pjrt.so"
    if not os.path.exists(so_path):
        return
    try:
        lib = ctypes.CDLL(so_path)
    except OSError:
        return
    if not hasattr(lib, "axon_start_nrt_profile"):
        return
    lib.axon_start_nrt_profile.argtypes = [
        ctypes.POINTER(ctypes.c_int64), ctypes.c_size_t]
    lib.axon_start_nrt_profile.restype = ctypes.c_int64
    lib.axon_stop_nrt_profile.argtypes = [ctypes.c_char_p]
    lib.axon_stop_nrt_profile.restype = ctypes.c_int64

    @contextlib.contextmanager
    def _hook(output_dir, device_ids):
        import jax
        jax.devices()  # force PJRT init so the .so's client exists
        if device_ids:
            ids = (ctypes.c_int64 * len(device_ids))(*device_ids)
            rc = lib.axon_start_nrt_profile(ids, len(device_ids))
        else:
            rc = lib.axon_start_nrt_profile(None, 0)
        if rc != 0:
            raise RuntimeError(f"axon_start_nrt_profile rc={rc}")
        try:
            yield
        finally:
            n = lib.axon_stop_nrt_profile(str(output_dir).encode())
            print(f"ntff profile: {n} file(s) -> {output_dir}", file=sys.stderr)

    import antenv
    mod = types.ModuleType("antenv.axon_hooks")
    mod.get_axon_ntff_profile_hook = lambda: _hook
    mod.set_axon_ntff_profile_hook = lambda h: None
    sys.modules["antenv.axon_hooks"] = mod
    antenv.axon_hooks = mod

B, S, D, F, E = 4, 2048, 1024, 4096, 8
T = B * S
TOP_K = 2
NCORES = 8
P = 128
ND, NF = D // P, F // P  # 8, 32

# test.py pokes these for profiling info
LAST_RESULT = None

_cache = {}


def _chunk_list(C):
    """Token chunks (PSUM free-dim <= 512, multiples of 128)."""
    chunks = [512] * (C // 512)
    if C % 512:
        chunks.append(C % 512)
    return chunks


def _build_bass(C, prec):
    dt = mybir.dt
    fp16_path = prec != "fp32"
    io_dt = dt.float16 if fp16_path else dt.float32
    nc = bacc.Bacc("TRN2", target_bir_lowering=False, debug=False)

    xT = nc.dram_tensor("xT", [D, C], io_dt, kind="ExternalInput")
    win = nc.dram_tensor("win", [D, F], io_dt, kind="ExternalInput")
    wout = nc.dram_tensor("wout", [F, D], io_dt, kind="ExternalInput")
    bin_ = nc.dram_tensor("bin", [F], dt.float32, kind="ExternalInput")
    bout = nc.dram_tensor("bout", [D], dt.float32, kind="ExternalInput")
    wcomb = nc.dram_tensor("wcomb", [P, C], dt.float32, kind="ExternalInput")
    yT = nc.dram_tensor("yT", [D, C], dt.float32, kind="ExternalOutput")

    xT_r = xT.ap().rearrange("(dn p) c -> p dn c", p=P)
    win_r = win.ap().rearrange("(dn p) f -> p dn f", p=P)
    wout_r = wout.ap().rearrange("(fn p) d -> p fn d", p=P)
    yT_r = yT.ap().rearrange("(dn p) c -> p dn c", p=P)

    chunks = _chunk_list(C)

    with tile.TileContext(nc) as tc, ExitStack() as ctx:
        consts = ctx.enter_context(tc.tile_pool(name="consts", bufs=1))
        xpool = ctx.enter_context(tc.tile_pool(name="x", bufs=2))
        winpool = ctx.enter_context(tc.tile_pool(name="win", bufs=3))
        woutpool = ctx.enter_context(tc.tile_pool(name="wout", bufs=1))
        hpool = ctx.enter_context(tc.tile_pool(name="h", bufs=1))
        ypool = ctx.enter_context(tc.tile_pool(name="y", bufs=4))
        psum_h = ctx.enter_context(tc.tile_pool(name="ph", bufs=4, space="PSUM"))
        psum_y = ctx.enter_context(tc.tile_pool(name="py", bufs=2, space="PSUM"))

        def x_dma(ck, csl):
            x_t = xpool.tile([P, ND, ck], io_dt, tag="x")
            nc.sync.dma_start(x_t[:], xT_r[:, :, csl])
            return x_t

        def win_dma(fo):
            win_t = winpool.tile([P, ND, 512], io_dt, tag="win")
            nc.sync.dma_start(win_t[:], win_r[:, :, fo * 512:(fo + 1) * 512])
            return win_t

        # critical path for the very first matmul: x chunk 0 + W_in stripe
        # 0 go FIRST on the Sync HWDGE queue so PE starts ~50us earlier.
        x0_t = x_dma(chunks[0], slice(0, chunks[0]))
        win0_t = win_dma(0)

        # bulk/background loads go on the other DMA queues so they don't
        # sit in front of the critical path: W_out on the Act HWDGE queue,
        # small constants on the GpSimd SWDGE queue.
        bin_t = consts.tile([P, NF], dt.float32)
        nc.gpsimd.dma_start(bin_t[:], bin_.ap().rearrange("(fo fi) -> fi fo", fi=P))
        bout_t = consts.tile([P, ND], dt.float32)
        nc.gpsimd.dma_start(bout_t[:], bout.ap().rearrange("(do di) -> di do", di=P))
        w_t = consts.tile([P, C], dt.float32)
        nc.gpsimd.dma_start(w_t[:], wcomb.ap())

        if fp16_path:
            # whole W_out resident in SBUF (8.4 MB fp16), loaded once
            wout_tiles = []
            for fo in range(8):
                wt = woutpool.tile([P, 4, D], io_dt, tag=f"wout{fo}")
                nc.scalar.dma_start(wt[:], wout_r[:, fo * 4:(fo + 1) * 4, :])
                wout_tiles.append(wt)

        off = 0
        for ci, ck in enumerate(chunks):
            csl = slice(off, off + ck)
            x_t = x0_t if ci == 0 else x_dma(ck, csl)

            # ---- phase A: h = gelu(W_in^T @ x + b_in), laid out [f, tok]
            h_t = hpool.tile([P, NF, ck], io_dt, tag="h")
            for fo in range(8):  # 512-wide stripes of F
                win_t = win0_t if (ci == 0 and fo == 0) else win_dma(fo)
                for j in range(4):
                    fc = fo * 4 + j
                    ph = psum_h.tile([P, ck], dt.float32, tag="ph")
                    for dn in range(ND):
                        nc.tensor.matmul(
                            ph[:],
                            win_t[:, dn, j * P:(j + 1) * P],
                            x_t[:, dn, :],
                            start=(dn == 0),
                            stop=(dn == ND - 1),
                        )
                    nc.scalar.activation(
                        h_t[:, fc, :], ph[:],
                        mybir.ActivationFunctionType.Gelu,
                        bias=bin_t[:, fc:fc + 1],
                    )

            # ---- phase B: y = w * (W_out^T @ h + b_out), laid out [d, tok]
            if fp16_path:
                for dn in range(ND):
                    py = psum_y.tile([P, ck], dt.float32, tag="py")
                    for fc in range(NF):
                        nc.tensor.matmul(
                            py[:],
                            wout_tiles[fc // 4][:, fc % 4, dn * P:(dn + 1) * P],
                            h_t[:, fc, :],
                            start=(fc == 0),
                            stop=(fc == NF - 1),
                        )
                    y_t = ypool.tile([P, ck], dt.float32, tag="y")
                    nc.scalar.activation(
                        y_t[:], py[:],
                        mybir.ActivationFunctionType.Identity,
                        bias=bout_t[:, dn:dn + 1],
                    )
                    nc.vector.tensor_mul(y_t[:], y_t[:], w_t[:, csl])
                    nc.scalar.dma_start(yT_r[:, dn, csl], y_t[:])
            else:
                # fp32: W_out too big to keep resident; stream it per chunk
                # in two d-halves (4 PSUM banks live per half).
                for dh in range(2):
                    pys = []
                    for i in range(4):
                        py = psum_y.tile([P, ck], dt.float32, tag=f"py{i}")
                        pys.append(py)
                    for fc in range(NF):
                        wt = woutpool.tile([P, 512], io_dt, tag="wouts")
                        nc.sync.dma_start(
                            wt[:], wout_r[:, fc, dh * 512:(dh + 1) * 512])
                        for i in range(4):
                            nc.tensor.matmul(
                                py := pys[i],
                                wt[:, i * P:(i + 1) * P],
                                h_t[:, fc, :],
                                start=(fc == 0),
                                stop=(fc == NF - 1),
                            )
                    for i in range(4):
                        dn = dh * 4 + i
                        y_t = ypool.tile([P, ck], dt.float32, tag="y")
                        nc.scalar.activation(
                            y_t[:], pys[i][:],
                            mybir.ActivationFunctionType.Identity,
                            bias=bout_t[:, dn:dn + 1],
                        )
                        nc.vector.tensor_mul(y_t[:], y_t[:], w_t[:, csl])
                        nc.sync.dma_start(yT_r[:, dn, csl], y_t[:])
            off += ck

    nc.compile()
    return nc


def _get_nc(C, prec):
    key = (C, prec)
    if key not in _cache:
        _cache[key] = _build_bass(C, prec)
    return _cache[key]


def _route(x, W_router):
    """Host-side router: top-2 selection + renormalized weights (fp64).

    Matches jax.lax.top_k on softmax(logits): softmax is monotone so
    top-2 of logits is identical, with ties broken toward lower index
    (argsort stable on -logits).
    """
    lg = x.astype(np.float64) @ W_router.T.astype(np.float64)
    top2 = np.argsort(-lg, axis=1, kind="stable")[:, :TOP_K]
    l1 = np.take_along_axis(lg, top2[:, 0:1], 1)
    l2 = np.take_along_axis(lg, top2[:, 1:2], 1)
    e2 = np.exp(l2 - l1)
    w1 = (1.0 / (1.0 + e2)).astype(np.float32)
    w2 = (e2 / (1.0 + e2)).astype(np.float32)
    return top2, np.concatenate([w1, w2], axis=1)


def kernel(residual, W_router, W_in, b_in, W_out, b_out):
    global LAST_RESULT
    prec = os.environ.get("MOE_PREC", "fp16")
    np_io = np.float16 if prec != "fp32" else np.float32

    x = np.ascontiguousarray(np.asarray(residual, dtype=np.float32).reshape(T, D))
    W_in = np.asarray(W_in, dtype=np.float32)
    W_out = np.asarray(W_out, dtype=np.float32)
    b_in = np.asarray(b_in, dtype=np.float32)
    b_out = np.asarray(b_out, dtype=np.float32)

    top2, wts = _route(x, np.asarray(W_router, dtype=np.float32))

    idxs, ws = [], []
    for e in range(E):
        sel0 = top2[:, 0] == e
        sel1 = top2[:, 1] == e
        idx = np.concatenate([np.where(sel0)[0], np.where(sel1)[0]])
        w = np.concatenate([wts[sel0, 0], wts[sel1, 1]])
        idxs.append(idx)
        ws.append(w)

    C = max(len(i) for i in idxs)
    C = ((C + P - 1) // P) * P
    nc = _get_nc(C, prec)

    xt = np.ascontiguousarray(x.T)  # [D, T]
    in_maps = []
    for e in range(E):
        cnt = len(idxs[e])
        xT_e = np.zeros((D, C), dtype=np_io)
        xT_e[:, :cnt] = xt[:, idxs[e]]
        wc_e = np.zeros((P, C), dtype=np.float32)
        wc_e[:, :cnt] = ws[e][None, :]
        in_maps.append({
            "xT": xT_e,
            "win": np.ascontiguousarray(W_in[e], dtype=np_io),
            "wout": np.ascontiguousarray(W_out[e], dtype=np_io),
            "bin": b_in[e],
            "bout": b_out[e],
            "wcomb": wc_e,
        })

    if os.environ.get("BASS_TRACE"):
        _install_ntff_hook()
    LAST_RESULT = run_bass_kernel_spmd(nc, in_maps, list(range(NCORES)))

    y = np.zeros((T, D), dtype=np.float32)
    for e in range(E):
        cnt = len(idxs[e])
        y[idxs[e]] += LAST_RESULT.results[e]["yT"][:, :cnt].T
    return y.reshape(B, S, D)


# revision 10
# speedup vs baseline: 1.0474x; 1.0247x over previous
"""MoE MLP (top-2 routing, 8 experts) on 8 Trainium2 NeuronCores.

Strategy (expert-parallel, per the sharding hint): each core owns one
expert's weights. The router (a [8,1024] matmul + softmax + top-2 —
0.05% of total FLOPs) runs on the host, which doubles as the dispatch
step: tokens are gathered per selected expert and shipped to that
expert's core, replacing the all-to-all. Each core runs a fused
gelu-MLP Bass kernel over its routed tokens:

    yT = w ⊙ (W_out^T @ gelu(W_in^T @ xT + b_in) + b_out)

in a transposed layout (tokens along the free axis) so both matmuls
keep the *weights* stationary on the PE array and no on-chip
transposes are needed anywhere. W_out stays resident in SBUF; W_in
streams once per token chunk. The host scatter-adds the per-expert
results back into the full [B,S,D] output.

Matmuls run in fp16 (same PE throughput as bf16 — 4x fp32 — but 8x
finer mantissa; measured end-to-end error vs the fp32 reference is
~4e-4 scale-relative). Set MOE_PREC=fp32 to force full fp32 matmuls.
"""

import contextlib
import ctypes
import os
import sys
import types
from contextlib import ExitStack

import numpy as np

import concourse.bass as bass
import concourse.mybir as mybir
import concourse.tile as tile
from concourse import bacc
from concourse.bass_utils import run_bass_kernel_spmd


def _install_ntff_hook():
    """Provide antenv.axon_hooks (absent in this image) so BASS_TRACE=1
    can capture NTFF profiles through the axon PJRT .so. No-op if the
    module already exists or the .so/symbols are unavailable."""
    try:
        from antenv.axon_hooks import get_axon_ntff_profile_hook  # noqa: F401
        return
    except ImportError:
        pass
    so_path = "/opt/axon/libaxon_pjrt.so"
    if not os.path.exists(so_path):
        return
    try:
        lib = ctypes.CDLL(so_path)
    except OSError:
        return
    if not hasattr(lib, "axon_start_nrt_profile"):
        return
    lib.axon_start_nrt_profile.argtypes = [
        ctypes.POINTER(ctypes.c_int64), ctypes.c_size_t]
    lib.axon_start_nrt_profile.restype = ctypes.c_int64
    lib.axon_stop_nrt_profile.argtypes = [ctypes.c_char_p]
    lib.axon_stop_nrt_profile.restype = ctypes.c_int64

    @contextlib.contextmanager
    def _hook(output_dir, device_ids):
        import jax
        jax.devices()  # force PJRT init so the .so's client exists
        if device_ids:
            ids = (ctypes.c_int64 * len(device_ids))(*device_ids)
            rc = lib.axon_start_nrt_profile(ids, len(device_ids))
        else:
            rc = lib.axon_start_nrt_profile(None, 0)
        if rc != 0:
            raise RuntimeError(f"axon_start_nrt_profile rc={rc}")
        try:
            yield
        finally:
            n = lib.axon_stop_nrt_profile(str(output_dir).encode())
            print(f"ntff profile: {n} file(s) -> {output_dir}", file=sys.stderr)

    import antenv
    mod = types.ModuleType("antenv.axon_hooks")
    mod.get_axon_ntff_profile_hook = lambda: _hook
    mod.set_axon_ntff_profile_hook = lambda h: None
    sys.modules["antenv.axon_hooks"] = mod
    antenv.axon_hooks = mod

B, S, D, F, E = 4, 2048, 1024, 4096, 8
T = B * S
TOP_K = 2
NCORES = 8
P = 128
ND, NF = D // P, F // P  # 8, 32

# test.py pokes these for profiling info
LAST_RESULT = None

_cache = {}


def _chunk_list(C):
    """Token chunks (PSUM free-dim <= 512, multiples of 128).

    Chunks below 256 run LDWEIGHTS-bound on the PE (weight load ~60ns
    vs a 53ns N=128 matmul), so a short tail is split off the previous
    512 chunk into two >=256 pieces instead.
    """
    chunks = [512] * (C // 512)
    rem = C % 512
    if rem:
        if rem < 256 and chunks:
            total = 512 + rem
            a = ((total // 2 + 127) // 128) * 128
            chunks[-1] = a
            chunks.append(total - a)
        else:
            chunks.append(rem)
    return chunks


def _build_bass(C, prec):
    dt = mybir.dt
    fp16_path = prec != "fp32"
    io_dt = dt.float16 if fp16_path else dt.float32
    nc = bacc.Bacc("TRN2", target_bir_lowering=False, debug=False)

    xT = nc.dram_tensor("xT", [D, C], io_dt, kind="ExternalInput")
    win = nc.dram_tensor("win", [D, F], io_dt, kind="ExternalInput")
    wout = nc.dram_tensor("wout", [F, D], io_dt, kind="ExternalInput")
    bin_ = nc.dram_tensor("bin", [F], dt.float32, kind="ExternalInput")
    bout = nc.dram_tensor("bout", [D], dt.float32, kind="ExternalInput")
    wcomb = nc.dram_tensor("wcomb", [P, C], dt.float32, kind="ExternalInput")
    yT = nc.dram_tensor("yT", [D, C], dt.float32, kind="ExternalOutput")

    xT_r = xT.ap().rearrange("(dn p) c -> p dn c", p=P)
    win_r = win.ap().rearrange("(dn p) f -> p dn f", p=P)
    wout_r = wout.ap().rearrange("(fn p) d -> p fn d", p=P)
    yT_r = yT.ap().rearrange("(dn p) c -> p dn c", p=P)

    chunks = _chunk_list(C)

    with tile.TileContext(nc) as tc, ExitStack() as ctx:
        consts = ctx.enter_context(tc.tile_pool(name="consts", bufs=1))
        xpool = ctx.enter_context(tc.tile_pool(name="x", bufs=2))
        winpool = ctx.enter_context(tc.tile_pool(name="win", bufs=3))
        woutpool = ctx.enter_context(tc.tile_pool(name="wout", bufs=1))
        hpool = ctx.enter_context(tc.tile_pool(name="h", bufs=1))
        ypool = ctx.enter_context(tc.tile_pool(name="y", bufs=4))
        psum_h = ctx.enter_context(tc.tile_pool(name="ph", bufs=4, space="PSUM"))
        psum_y = ctx.enter_context(tc.tile_pool(name="py", bufs=2, space="PSUM"))

        def x_dma(ck, csl):
            x_t = xpool.tile([P, ND, ck], io_dt, tag="x")
            nc.sync.dma_start(x_t[:], xT_r[:, :, csl])
            return x_t

        def win_dma(fo):
            win_t = winpool.tile([P, ND, 512], io_dt, tag="win")
            nc.sync.dma_start(win_t[:], win_r[:, :, fo * 512:(fo + 1) * 512])
            return win_t

        # critical path for the very first matmul: x chunk 0 + W_in stripe
        # 0 go FIRST on the Sync HWDGE queue so PE starts ~50us earlier.
        x0_t = x_dma(chunks[0], slice(0, chunks[0]))
        win0_t = win_dma(0)

        # b_in is needed by the first gelu; it's tiny — SWDGE queue.
        bin_t = consts.tile([P, NF], dt.float32)
        nc.gpsimd.dma_start(bin_t[:], bin_.ap().rearrange("(fo fi) -> fi fo", fi=P))

        # PE HAM warm-up: ~3us of junk matmuls on a scratch tile while the
        # x0/win0 DMAs are in flight, so real matmuls start at 2.4 GHz
        # instead of spending the first activity window at 1.2 GHz.
        wu_t = consts.tile([P, P], io_dt)
        nc.gpsimd.memset(wu_t[:], 0.0)
        wu_ps = ctx.enter_context(tc.tile_pool(name="wups", bufs=1, space="PSUM"))
        wu_p = wu_ps.tile([P, 64], dt.float32)
        for _ in range(36):
            nc.tensor.matmul(wu_p[:], wu_t[:], wu_t[:, :64], start=True, stop=True)

        # remaining background loads are emitted inside chunk 0's phase A
        # (below) so they trail the critical path on their queues instead
        # of stealing HBM bandwidth from it.
        bout_t = consts.tile([P, ND], dt.float32)
        w_t = consts.tile([P, C], dt.float32)
        wout_tiles = []
        if fp16_path:
            for fo in range(8):
                wout_tiles.append(
                    woutpool.tile([P, 4, D], io_dt,
                                  tag=f"wout{fo}", name=f"wout{fo}"))

        off = 0
        for ci, ck in enumerate(chunks):
            csl = slice(off, off + ck)
            x_t = x0_t if ci == 0 else x_dma(ck, csl)

            # ---- phase A: h = gelu(W_in^T @ x + b_in), laid out [f, tok]
            h_t = hpool.tile([P, NF, ck], io_dt, tag="h")
            for fo in range(8):  # 512-wide stripes of F
                win_t = win0_t if (ci == 0 and fo == 0) else win_dma(fo)
                for j in range(4):
                    fc = fo * 4 + j
                    ph = psum_h.tile([P, ck], dt.float32, tag="ph")
                    for dn in range(ND):
                        nc.tensor.matmul(
                            ph[:],
                            win_t[:, dn, j * P:(j + 1) * P],
                            x_t[:, dn, :],
                            start=(dn == 0),
                            stop=(dn == ND - 1),
                        )
                    nc.scalar.activation(
                        h_t[:, fc, :], ph[:],
                        mybir.ActivationFunctionType.Gelu,
                        bias=bin_t[:, fc:fc + 1],
                    )
                if ci == 0:
                    # trail the bulk W_out load (Act HWDGE queue) behind
                    # this chunk's gelus; it's only needed in phase B.
                    if fp16_path:
                        nc.scalar.dma_start(
                            wout_tiles[fo][:],
                            wout_r[:, fo * 4:(fo + 1) * 4, :])
                    if fo == 0:
                        nc.gpsimd.dma_start(
                            bout_t[:],
                            bout.ap().rearrange("(do di) -> di do", di=P))
                    elif fo == 1:
                        nc.gpsimd.dma_start(w_t[:], wcomb.ap())

            # ---- phase B: y = w * (W_out^T @ h + b_out), laid out [d, tok]
            if fp16_path:
                for dn in range(ND):
                    py = psum_y.tile([P, ck], dt.float32, tag="py")
                    for fc in range(NF):
                        nc.tensor.matmul(
                            py[:],
                            wout_tiles[fc // 4][:, fc % 4, dn * P:(dn + 1) * P],
                            h_t[:, fc, :],
                            start=(fc == 0),
                            stop=(fc == NF - 1),
                        )
                    y_t = ypool.tile([P, ck], dt.float32, tag="y")
                    nc.scalar.activation(
                        y_t[:], py[:],
                        mybir.ActivationFunctionType.Identity,
                        bias=bout_t[:, dn:dn + 1],
                    )
                    nc.vector.tensor_mul(y_t[:], y_t[:], w_t[:, csl])
                    nc.scalar.dma_start(yT_r[:, dn, csl], y_t[:])
            else:
                # fp32: W_out too big to keep resident; stream it per chunk
                # in two d-halves (4 PSUM banks live per half).
                for dh in range(2):
                    pys = []
                    for i in range(4):
                        py = psum_y.tile([P, ck], dt.float32, tag=f"py{i}")
                        pys.append(py)
                    for fc in range(NF):
                        wt = woutpool.tile([P, 512], io_dt, tag="wouts")
                        nc.sync.dma_start(
                            wt[:], wout_r[:, fc, dh * 512:(dh + 1) * 512])
                        for i in range(4):
                            nc.tensor.matmul(
                                py := pys[i],
                                wt[:, i * P:(i + 1) * P],
                                h_t[:, fc, :],
                                start=(fc == 0),
                                stop=(fc == NF - 1),
                            )
                    for i in range(4):
                        dn = dh * 4 + i
                        y_t = ypool.tile([P, ck], dt.float32, tag="y")
                        nc.scalar.activation(
                            y_t[:], pys[i][:],
                            mybir.ActivationFunctionType.Identity,
                            bias=bout_t[:, dn:dn + 1],
                        )
                        nc.vector.tensor_mul(y_t[:], y_t[:], w_t[:, csl])
                        nc.sync.dma_start(yT_r[:, dn, csl], y_t[:])
            off += ck

    nc.compile()
    return nc


def _get_nc(C, prec):
    key = (C, prec)
    if key not in _cache:
        _cache[key] = _build_bass(C, prec)
    return _cache[key]


def _route(x, W_router):
    """Host-side router: top-2 selection + renormalized weights (fp64).

    Matches jax.lax.top_k on softmax(logits): softmax is monotone so
    top-2 of logits is identical, with ties broken toward lower index
    (argsort stable on -logits).
    """
    lg = x.astype(np.float64) @ W_router.T.astype(np.float64)
    top2 = np.argsort(-lg, axis=1, kind="stable")[:, :TOP_K]
    l1 = np.take_along_axis(lg, top2[:, 0:1], 1)
    l2 = np.take_along_axis(lg, top2[:, 1:2], 1)
    e2 = np.exp(l2 - l1)
    w1 = (1.0 / (1.0 + e2)).astype(np.float32)
    w2 = (e2 / (1.0 + e2)).astype(np.float32)
    return top2, np.concatenate([w1, w2], axis=1)


def kernel(residual, W_router, W_in, b_in, W_out, b_out):
    global LAST_RESULT
    prec = os.environ.get("MOE_PREC", "fp16")
    np_io = np.float16 if prec != "fp32" else np.float32

    x = np.ascontiguousarray(np.asarray(residual, dtype=np.float32).reshape(T, D))
    W_in = np.asarray(W_in, dtype=np.float32)
    W_out = np.asarray(W_out, dtype=np.float32)
    b_in = np.asarray(b_in, dtype=np.float32)
    b_out = np.asarray(b_out, dtype=np.float32)

    top2, wts = _route(x, np.asarray(W_router, dtype=np.float32))

    idxs, ws = [], []
    for e in range(E):
        sel0 = top2[:, 0] == e
        sel1 = top2[:, 1] == e
        idx = np.concatenate([np.where(sel0)[0], np.where(sel1)[0]])
        w = np.concatenate([wts[sel0, 0], wts[sel1, 1]])
        idxs.append(idx)
        ws.append(w)

    C = max(len(i) for i in idxs)
    C = ((C + P - 1) // P) * P
    nc = _get_nc(C, prec)

    xt = np.ascontiguousarray(x.T)  # [D, T]
    in_maps = []
    for e in range(E):
        cnt = len(idxs[e])
        xT_e = np.zeros((D, C), dtype=np_io)
        xT_e[:, :cnt] = xt[:, idxs[e]]
        wc_e = np.zeros((P, C), dtype=np.float32)
        wc_e[:, :cnt] = ws[e][None, :]
        in_maps.append({
            "xT": xT_e,
            "win": np.ascontiguousarray(W_in[e], dtype=np_io),
            "wout": np.ascontiguousarray(W_out[e], dtype=np_io),
            "bin": b_in[e],
            "bout": b_out[e],
            "wcomb": wc_e,
        })

    if os.environ.get("BASS_TRACE"):
        _install_ntff_hook()
    LAST_RESULT = run_bass_kernel_spmd(nc, in_maps, list(range(NCORES)))

    y = np.zeros((T, D), dtype=np.float32)
    for e in range(E):
        cnt = len(idxs[e])
        y[idxs[e]] += LAST_RESULT.results[e]["yT"][:, :cnt].T
    return y.reshape(B, S, D)


# revision 13
# speedup vs baseline: 1.0671x; 1.0188x over previous
"""MoE MLP (top-2 routing, 8 experts) on 8 Trainium2 NeuronCores.

Strategy (expert-parallel, per the sharding hint): each core owns one
expert's weights. The router (a [8,1024] matmul + softmax + top-2 —
0.05% of total FLOPs) runs on the host, which doubles as the dispatch
step: tokens are gathered per selected expert and shipped to that
expert's core, replacing the all-to-all. Each core runs a fused
gelu-MLP Bass kernel over its routed tokens:

    yT = w ⊙ (W_out^T @ gelu(W_in^T @ xT + b_in) + b_out)

in a transposed layout (tokens along the free axis) so both matmuls
keep the *weights* stationary on the PE array and no on-chip
transposes are needed anywhere. W_out stays resident in SBUF; W_in
streams once per token chunk. The host scatter-adds the per-expert
results back into the full [B,S,D] output.

Matmuls run in fp16 (same PE throughput as bf16 — 4x fp32 — but 8x
finer mantissa; measured end-to-end error vs the fp32 reference is
~4e-4 scale-relative). Set MOE_PREC=fp32 to force full fp32 matmuls.
"""

import contextlib
import ctypes
import os
import sys
import types
from contextlib import ExitStack

import numpy as np

import concourse.bass as bass
import concourse.mybir as mybir
import concourse.tile as tile
from concourse import bacc
from concourse.bass_utils import run_bass_kernel_spmd


def _install_ntff_hook():
    """Provide antenv.axon_hooks (absent in this image) so BASS_TRACE=1
    can capture NTFF profiles through the axon PJRT .so. No-op if the
    module already exists or the .so/symbols are unavailable."""
    try:
        from antenv.axon_hooks import get_axon_ntff_profile_hook  # noqa: F401
        return
    except ImportError:
        pass
    so_path = "/opt/axon/libaxon_pjrt.so"
    if not os.path.exists(so_path):
        return
    try:
        lib = ctypes.CDLL(so_path)
    except OSError:
        return
    if not hasattr(lib, "axon_start_nrt_profile"):
        return
    lib.axon_start_nrt_profile.argtypes = [
        ctypes.POINTER(ctypes.c_int64), ctypes.c_size_t]
    lib.axon_start_nrt_profile.restype = ctypes.c_int64
    lib.axon_stop_nrt_profile.argtypes = [ctypes.c_char_p]
    lib.axon_stop_nrt_profile.restype = ctypes.c_int64

    @contextlib.contextmanager
    def _hook(output_dir, device_ids):
        import jax
        jax.devices()  # force PJRT init so the .so's client exists
        if device_ids:
            ids = (ctypes.c_int64 * len(device_ids))(*device_ids)
            rc = lib.axon_start_nrt_profile(ids, len(device_ids))
        else:
            rc = lib.axon_start_nrt_profile(None, 0)
        if rc != 0:
            raise RuntimeError(f"axon_start_nrt_profile rc={rc}")
        try:
            yield
        finally:
            n = lib.axon_stop_nrt_profile(str(output_dir).encode())
            print(f"ntff profile: {n} file(s) -> {output_dir}", file=sys.stderr)

    import antenv
    mod = types.ModuleType("antenv.axon_hooks")
    mod.get_axon_ntff_profile_hook = lambda: _hook
    mod.set_axon_ntff_profile_hook = lambda h: None
    sys.modules["antenv.axon_hooks"] = mod
    antenv.axon_hooks = mod

B, S, D, F, E = 4, 2048, 1024, 4096, 8
T = B * S
TOP_K = 2
NCORES = 8
P = 128
ND, NF = D // P, F // P  # 8, 32

# test.py pokes these for profiling info
LAST_RESULT = None

_cache = {}


def _chunk_list(C):
    """Token chunks (PSUM free-dim <= 512, multiples of 128).

    Chunks below 256 run LDWEIGHTS-bound on the PE (weight load ~60ns
    vs a 53ns N=128 matmul), so a short tail is split off the previous
    512 chunk into two >=256 pieces instead.
    """
    chunks = [512] * (C // 512)
    rem = C % 512
    if rem:
        if rem < 256 and chunks:
            total = 512 + rem
            a = ((total // 2 + 127) // 128) * 128
            chunks[-1] = a
            chunks.append(total - a)
        else:
            chunks.append(rem)
    return chunks


def _build_bass(C, prec):
    dt = mybir.dt
    fp16_path = prec != "fp32"
    io_dt = dt.float16 if fp16_path else dt.float32
    nc = bacc.Bacc("TRN2", target_bir_lowering=False, debug=False)

    xT = nc.dram_tensor("xT", [D, C], io_dt, kind="ExternalInput")
    win = nc.dram_tensor("win", [D, F], io_dt, kind="ExternalInput")
    wout = nc.dram_tensor("wout", [F, D], io_dt, kind="ExternalInput")
    bin_ = nc.dram_tensor("bin", [F], dt.float32, kind="ExternalInput")
    bout = nc.dram_tensor("bout", [D], dt.float32, kind="ExternalInput")
    wcomb = nc.dram_tensor("wcomb", [P, C], dt.float32, kind="ExternalInput")
    yT = nc.dram_tensor("yT", [D, C], dt.float32, kind="ExternalOutput")

    xT_r = xT.ap().rearrange("(dn p) c -> p dn c", p=P)
    win_r = win.ap().rearrange("(dn p) f -> p dn f", p=P)
    wout_r = wout.ap().rearrange("(fn p) d -> p fn d", p=P)
    yT_r = yT.ap().rearrange("(dn p) c -> p dn c", p=P)

    chunks = _chunk_list(C)

    with tile.TileContext(nc) as tc, ExitStack() as ctx:
        consts = ctx.enter_context(tc.tile_pool(name="consts", bufs=1))
        xpool = ctx.enter_context(tc.tile_pool(name="x", bufs=2))
        winpool = ctx.enter_context(tc.tile_pool(name="win", bufs=3))
        woutpool = ctx.enter_context(tc.tile_pool(name="wout", bufs=1))
        hpool = ctx.enter_context(tc.tile_pool(name="h", bufs=1))
        ypool = ctx.enter_context(tc.tile_pool(name="y", bufs=4))
        psum_h = ctx.enter_context(tc.tile_pool(name="ph", bufs=4, space="PSUM"))
        psum_y = ctx.enter_context(tc.tile_pool(name="py", bufs=2, space="PSUM"))

        def x_dma(ck, csl):
            x_t = xpool.tile([P, ND, ck], io_dt, tag="x")
            nc.sync.dma_start(x_t[:], xT_r[:, :, csl])
            return x_t

        def win_dma(fo):
            win_t = winpool.tile([P, ND, 512], io_dt, tag="win")
            nc.sync.dma_start(win_t[:], win_r[:, :, fo * 512:(fo + 1) * 512])
            return win_t

        # critical path for the very first matmul: x chunk 0 + W_in stripe
        # 0 go FIRST on the Sync HWDGE queue so PE starts ~50us earlier.
        x0_t = x_dma(chunks[0], slice(0, chunks[0]))
        win0_t = win_dma(0)

        # b_in is needed by the first gelu; it's tiny — SWDGE queue.
        bin_t = consts.tile([P, NF], dt.float32)
        nc.gpsimd.dma_start(bin_t[:], bin_.ap().rearrange("(fo fi) -> fi fo", fi=P))

        # PE HAM warm-up: ~3us of junk matmuls on a scratch tile while the
        # x0/win0 DMAs are in flight, so real matmuls start at 2.4 GHz
        # instead of spending the first activity window at 1.2 GHz.
        wu_t = consts.tile([P, P], io_dt)
        nc.gpsimd.memset(wu_t[:], 0.0)
        wu_ps = ctx.enter_context(tc.tile_pool(name="wups", bufs=1, space="PSUM"))
        wu_p = wu_ps.tile([P, 64], dt.float32)
        for _ in range(36):
            nc.tensor.matmul(wu_p[:], wu_t[:], wu_t[:, :64], start=True, stop=True)

        # Remaining bulk loads share the Sync HWDGE queue with the W_in
        # stripes, hand-interleaved below so each arrives just in time:
        # the queue drains in emission order, so wout stripe k loads
        # during phase-A stripe k's ~7us of matmuls and the whole of
        # W_out is resident right when phase B first needs it. (Putting
        # them on another queue doesn't work: the scheduler hoists
        # ready DMA triggers, and they'd steal HBM bandwidth from the
        # critical x0/win0 loads.)
        bout_t = consts.tile([P, ND], dt.float32)
        w_t = consts.tile([P, C], dt.float32)
        wout_tiles = []
        if fp16_path:
            for fo in range(8):
                wout_tiles.append(
                    woutpool.tile([P, 4, D], io_dt,
                                  tag=f"wout{fo}", name=f"wout{fo}"))

        off = 0
        for ci, ck in enumerate(chunks):
            csl = slice(off, off + ck)
            x_t = x0_t if ci == 0 else x_dma(ck, csl)

            # ---- phase A: h = gelu(W_in^T @ x + b_in), laid out [f, tok]
            h_t = hpool.tile([P, NF, ck], io_dt, tag="h")
            for fo in range(8):  # 512-wide stripes of F
                win_t = win0_t if (ci == 0 and fo == 0) else win_dma(fo)
                for j in range(4):
                    fc = fo * 4 + j
                    ph = psum_h.tile([P, ck], dt.float32, tag="ph")
                    for dn in range(ND):
                        nc.tensor.matmul(
                            ph[:],
                            win_t[:, dn, j * P:(j + 1) * P],
                            x_t[:, dn, :],
                            start=(dn == 0),
                            stop=(dn == ND - 1),
                        )
                    nc.scalar.activation(
                        h_t[:, fc, :], ph[:],
                        mybir.ActivationFunctionType.Gelu,
                        bias=bin_t[:, fc:fc + 1],
                    )
                if ci == 0:
                    if fp16_path:
                        nc.sync.dma_start(
                            wout_tiles[fo][:],
                            wout_r[:, fo * 4:(fo + 1) * 4, :])
                    if fo == 3:
                        nc.sync.dma_start(
                            bout_t[:],
                            bout.ap().rearrange("(do di) -> di do", di=P))
                    elif fo == 5:
                        nc.sync.dma_start(w_t[:], wcomb.ap())

            # ---- phase B: y = w * (W_out^T @ h + b_out), laid out [d, tok]
            if fp16_path:
                for dn in range(ND):
                    py = psum_y.tile([P, ck], dt.float32, tag="py")
                    for fc in range(NF):
                        nc.tensor.matmul(
                            py[:],
                            wout_tiles[fc // 4][:, fc % 4, dn * P:(dn + 1) * P],
                            h_t[:, fc, :],
                            start=(fc == 0),
                            stop=(fc == NF - 1),
                        )
                    y_t = ypool.tile([P, ck], dt.float32, tag="y")
                    # one DVE op: (psum + b_out) * w — keeps ScalarE on
                    # gelu only (no ACT table switching per chunk)
                    nc.vector.scalar_tensor_tensor(
                        y_t[:], py[:], bout_t[:, dn:dn + 1], w_t[:, csl],
                        op0=mybir.AluOpType.add, op1=mybir.AluOpType.mult,
                    )
                    nc.scalar.dma_start(yT_r[:, dn, csl], y_t[:])
            else:
                # fp32: W_out too big to keep resident; stream it per chunk
                # in two d-halves (4 PSUM banks live per half).
                for dh in range(2):
                    pys = []
                    for i in range(4):
                        py = psum_y.tile([P, ck], dt.float32, tag=f"py{i}")
                        pys.append(py)
                    for fc in range(NF):
                        wt = woutpool.tile([P, 512], io_dt, tag="wouts")
                        nc.sync.dma_start(
                            wt[:], wout_r[:, fc, dh * 512:(dh + 1) * 512])
                        for i in range(4):
                            nc.tensor.matmul(
                                py := pys[i],
                                wt[:, i * P:(i + 1) * P],
                                h_t[:, fc, :],
                                start=(fc == 0),
                                stop=(fc == NF - 1),
                            )
                    for i in range(4):
                        dn = dh * 4 + i
                        y_t = ypool.tile([P, ck], dt.float32, tag="y")
                        nc.scalar.activation(
                            y_t[:], pys[i][:],
                            mybir.ActivationFunctionType.Identity,
                            bias=bout_t[:, dn:dn + 1],
                        )
                        nc.vector.tensor_mul(y_t[:], y_t[:], w_t[:, csl])
                        nc.sync.dma_start(yT_r[:, dn, csl], y_t[:])
            off += ck

    nc.compile()
    return nc


def _get_nc(C, prec):
    key = (C, prec)
    if key not in _cache:
        _cache[key] = _build_bass(C, prec)
    return _cache[key]


def _route(x, W_router):
    """Host-side router: top-2 selection + renormalized weights (fp64).

    Matches jax.lax.top_k on softmax(logits): softmax is monotone so
    top-2 of logits is identical, with ties broken toward lower index
    (argsort stable on -logits).
    """
    lg = x.astype(np.float64) @ W_router.T.astype(np.float64)
    top2 = np.argsort(-lg, axis=1, kind="stable")[:, :TOP_K]
    l1 = np.take_along_axis(lg, top2[:, 0:1], 1)
    l2 = np.take_along_axis(lg, top2[:, 1:2], 1)
    e2 = np.exp(l2 - l1)
    w1 = (1.0 / (1.0 + e2)).astype(np.float32)
    w2 = (e2 / (1.0 + e2)).astype(np.float32)
    return top2, np.concatenate([w1, w2], axis=1)


def kernel(residual, W_router, W_in, b_in, W_out, b_out):
    global LAST_RESULT
    prec = os.environ.get("MOE_PREC", "fp16")
    np_io = np.float16 if prec != "fp32" else np.float32

    x = np.ascontiguousarray(np.asarray(residual, dtype=np.float32).reshape(T, D))
    W_in = np.asarray(W_in, dtype=np.float32)
    W_out = np.asarray(W_out, dtype=np.float32)
    b_in = np.asarray(b_in, dtype=np.float32)
    b_out = np.asarray(b_out, dtype=np.float32)

    top2, wts = _route(x, np.asarray(W_router, dtype=np.float32))

    idxs, ws = [], []
    for e in range(E):
        sel0 = top2[:, 0] == e
        sel1 = top2[:, 1] == e
        idx = np.concatenate([np.where(sel0)[0], np.where(sel1)[0]])
        w = np.concatenate([wts[sel0, 0], wts[sel1, 1]])
        idxs.append(idx)
        ws.append(w)

    C = max(len(i) for i in idxs)
    C = ((C + P - 1) // P) * P
    nc = _get_nc(C, prec)

    xt = np.ascontiguousarray(x.T)  # [D, T]
    in_maps = []
    for e in range(E):
        cnt = len(idxs[e])
        xT_e = np.zeros((D, C), dtype=np_io)
        xT_e[:, :cnt] = xt[:, idxs[e]]
        wc_e = np.zeros((P, C), dtype=np.float32)
        wc_e[:, :cnt] = ws[e][None, :]
        in_maps.append({
            "xT": xT_e,
            "win": np.ascontiguousarray(W_in[e], dtype=np_io),
            "wout": np.ascontiguousarray(W_out[e], dtype=np_io),
            "bin": b_in[e],
            "bout": b_out[e],
            "wcomb": wc_e,
        })

    if os.environ.get("BASS_TRACE"):
        _install_ntff_hook()
    LAST_RESULT = run_bass_kernel_spmd(nc, in_maps, list(range(NCORES)))

    y = np.zeros((T, D), dtype=np.float32)
    for e in range(E):
        cnt = len(idxs[e])
        y[idxs[e]] += LAST_RESULT.results[e]["yT"][:, :cnt].T
    return y.reshape(B, S, D)
